# revision 65
# baseline (speedup 1.0000x reference)
"""Trainium2 kernel for nn_ActorNetwork (8-branch graph-attention MLP).

Sharding: pure data parallelism. 8192 samples -> 8 cores x 1024.
Layer-major schedule; all per-tile intermediates resident in SBUF.
Per layer: PE transposes z -> zf (feature-major), gate matmul + batched
softmax, neighbor mix as scalar_tensor_tensor split across DVE and
GpSimd, PE transposes of mixed x -> xf, per-branch main matmuls into
PSUM, LayerNorm stats via bn_stats straight off PSUM, ReLU+affine on
ACT reading PSUM. PSUM->SBUF transpose copies rotate across
ACT/DVE/GpSimd. ACT table thrash avoided by batching exp per layer and
keeping only Copy/Relu (table-free) between the per-tile Sqrt calls.
"""

import numpy as np

import concourse.bass as bass
import concourse.mybir as mybir
import concourse.tile as tile
from concourse.bass_utils import run_bass_kernel_spmd

B = 8192
NCORES = 8
BL = B // NCORES          # 1024 samples per core
P = 128                   # partition tile (samples)
NT = BL // P              # 8 batch tiles per core
HID = [256, 128, 64]
OUT = 5
EPS = 1e-5
D0 = 4                    # layer-0 feature dim padded 3 -> 4

NEIGH = np.array([[j for j in range(8) if j != i] for i in range(8)])
_G = [
    [(0, 1), (0, 2), (0, 3), (4, 0), (5, 0), (6, 0), (7, 0)],
    [(0, 1), (2, 1), (3, 1), (4, 1), (5, 1), (6, 1), (7, 1)],
    [(0, 2), (1, 2), (3, 2), (4, 2), (5, 2), (6, 2), (7, 2)],
    [(0, 3), (1, 3), (2, 3), (4, 3), (5, 3), (6, 3), (7, 3)],
    [(0, 4), (1, 4), (2, 4), (3, 4), (5, 4), (6, 4), (7, 4)],
    [(0, 5), (1, 5), (2, 5), (3, 5), (4, 5), (6, 5), (7, 5)],
    [(0, 6), (1, 6), (2, 6), (3, 6), (4, 6), (5, 6), (7, 6)],
    [(0, 7), (1, 7), (2, 7), (3, 7), (4, 7), (5, 7), (6, 7)],
]
GIDX = np.array([[r * 8 + c for (r, c) in row] for row in _G]).reshape(-1)

F32 = mybir.dt.float32
BF16 = mybir.dt.bfloat16
AF = mybir.ActivationFunctionType
ALU = mybir.AluOpType

MIX_POOL_MIN_I = 4        # branches >= this go to GpSimd in the mix

# The tail drain accumulates one sync-wait per live semaphore lane; walrus
# rejects >1 wait on the Drain struct. Split the waits across SP nops, one
# lane each, before the real drain runs (it then has nothing left to wait on).
_orig_dab = tile.TileContext._drain_and_barrier


def _patched_dab(self, tick_clock, wait_clock):
    from concourse.vector_clock import ScopedClock, VectorClock

    gc = tick_clock.global_clock
    nproc = len(list(gc))
    for proc in range(nproc):
        val = gc[proc]
        if val <= 0:
            continue
        v = VectorClock()
        v.require_at_least(proc, val)
        nop = self.nc.sync.nop(nofuse=True)
        wait_clock.add_sem_waits(nop.ins, ScopedClock({None: v}))
    self.nc.sync.drain()
    self.nc.all_engine_barrier()
    assert self.sems is not None
    popped = self.nc._tile_sem_poison_stack.pop()
    assert popped is self._sem_poison
    self.nc.clear_and_free_semaphores(list(self.sems.allocated().values()))
    self.nc.all_engine_barrier()


tile.TileContext._drain_and_barrier = _patched_dab


def _bcast_last(ap, n):
    """Append a stride-0 innermost dim of size n to an AP (free-dim bcast)."""
    return bass.AP(tensor=ap.tensor, offset=ap.offset, ap=[*ap.ap, [0, n]])


def build_nc():
    nc = bass.Bass()
    DIMS = [D0, 256, 128]            # per-layer input dims (padded l0)
    EDIMS = [256, 128, 64]           # per-layer output dims
    KC = [1, 16, 8]                  # feature-major chunks of z per layer

    # ---- dram parameters ----
    z0_ext = nc.declare_dram_parameter("z0", [BL, 8 * D0], BF16, isOutput=False)
    gates_ext = nc.declare_dram_parameter("gates", [BL, 56], F32, isOutput=False)
    ident_ext = nc.declare_dram_parameter("ident", [P, P], BF16, isOutput=False)
    wgs1_ext = nc.declare_dram_parameter("wgs1", [16, 128, 56], BF16, isOutput=False)
    wgs2_ext = nc.declare_dram_parameter("wgs2", [8, 128, 56], BF16, isOutput=False)
    w0_ext = nc.declare_dram_parameter("w0", [8, 8 * D0, 256], BF16, isOutput=False)
    w1_ext = nc.declare_dram_parameter("w1", [8, 2, 128, 128], BF16, isOutput=False)
    w2_ext = nc.declare_dram_parameter("w2", [8, 128, 64], BF16, isOutput=False)
    wo_ext = nc.declare_dram_parameter("wo", [4, 128, 16], BF16, isOutput=False)
    out_ext = nc.declare_dram_parameter("out", [BL, 40], F32, isOutput=True)

    with tile.TileContext(nc) as tc:
        with (
            tc.tile_pool(name="singles", bufs=1) as singles,
            tc.tile_pool(name="ff", bufs=2) as ffp,
            tc.tile_pool(name="pt", bufs=2, space="PSUM") as ptp,
            tc.tile_pool(name="pg", bufs=2, space="PSUM") as pgp,
            tc.tile_pool(name="ph", bufs=2, space="PSUM") as php,
        ):
            # ---------- constants / weights ----------
            ident = singles.tile([P, P], BF16)
            nc.gpsimd.dma_start(out=ident, in_=ident_ext[:, :])
            eps_t = singles.tile([P, 1], F32)
            nc.vector.memset(eps_t, EPS)

            wgs1_sb = singles.tile([128, 16, 56], BF16)
            nc.gpsimd.dma_start(out=wgs1_sb, in_=wgs1_ext.rearrange("k p f -> p k f"))
            wgs2_sb = singles.tile([128, 8, 56], BF16)
            nc.gpsimd.dma_start(out=wgs2_sb, in_=wgs2_ext.rearrange("k p f -> p k f"))
            w0_sb = singles.tile([8 * D0, 8, 256], BF16)
            nc.gpsimd.dma_start(out=w0_sb, in_=w0_ext.rearrange("i p f -> p i f"))
            w1_sb = singles.tile([128, 8, 2, 128], BF16)
            nc.gpsimd.dma_start(out=w1_sb, in_=w1_ext.rearrange("i k p f -> p i k f"))
            w2_sb = singles.tile([128, 8, 64], BF16)
            nc.gpsimd.dma_start(out=w2_sb, in_=w2_ext.rearrange("i p f -> p i f"))
            wo_sb = singles.tile([128, 4, 16], BF16)
            nc.gpsimd.dma_start(out=wo_sb, in_=wo_ext.rearrange("c p f -> p c f"))

            # ---------- activations ----------
            zb0 = singles.tile([P, NT, 8, D0], BF16)
            nc.gpsimd.dma_start(
                out=zb0.rearrange("p t a b -> p t (a b)"),
                in_=z0_ext.rearrange("(t p) f -> p t f", p=P),
            )
            gates_sb = singles.tile([P, NT, 56], F32)
            nc.gpsimd.dma_start(
                out=gates_sb, in_=gates_ext.rearrange("(t p) f -> p t f", p=P)
            )

            z1 = singles.tile([P, NT, 8, 256], BF16)
            z2 = singles.tile([P, NT, 8, 128], BF16)
            z3 = singles.tile([P, NT, 8, 64], BF16)
            xf0 = singles.tile([8 * D0, NT, P], BF16)
            xb1 = singles.tile([P, NT, 8, 256], BF16)
            xb2 = singles.tile([P, NT, 8, 128], BF16)

            e_sb = singles.tile([P, NT, 56], F32)
            cc = singles.tile([P, NT, 56], F32)
            s8 = singles.tile([P, NT, 8], F32)
            sr8 = singles.tile([P, NT, 8], F32)
            stats = singles.tile([P, NT, 8, 6], F32)
            t1v = singles.tile([P, NT, 8], F32)
            t2v = singles.tile([P, NT, 8], F32)
            t3v = singles.tile([P, NT, 8], F32)
            varv = singles.tile([P, NT, 8], F32)
            stdv = singles.tile([P, NT, 8], F32)
            rstd = singles.tile([P, NT, 8], F32)
            nmr = singles.tile([P, NT, 8], F32)
            scr_a = singles.tile([2, 2], F32)
            scr_p1 = singles.tile([2, 2], F32)
            scr_p2 = singles.tile([2, 2], BF16)
            scr_v = singles.tile([2, 2], BF16)
            osb = singles.tile([P, NT, 64], F32)
            out_sb = singles.tile([P, NT, 40], F32)

            _eng_ctr = [0]

            def copy_eng(dst, src):
                # All PSUM->SBUF copies on ACT so every consumer's psum-WAR
                # and copy-RAW deps collapse onto the single Activation lane.
                nc.scalar.activation(dst, src, AF.Copy)

            def transpose_to(dstf, flat, ncols):
                """PE-transpose flat [128, ncols*128] into dstf chunks
                [ncols, 128]: per-chunk psum tile + ACT copy (the exact
                structure the scheduler handles within its one-wait-per-
                instruction budget)."""
                for c in range(ncols):
                    pt = ptp.tile([P, P], BF16, tag="pt")
                    nc.tensor.transpose(pt, flat[:, c * P : (c + 1) * P], ident)
                    copy_eng(dstf[:, c], pt)

            _z3f = [None]

            def head_tile(t):
                z3f = _z3f[0]
                flat = z3[:, t].rearrange("p a b -> p (a b)")
                transpose_to(z3f[:, t], flat, 4)
                po = pgp.tile([P, 64], F32, tag="pg")
                for c in range(4):
                    nc.tensor.matmul(
                        po[:, c * 16 : (c + 1) * 16],
                        z3f[:, t, c],
                        wo_sb[:, c],
                        start=True,
                        stop=True,
                    )
                nc.scalar.activation(osb[:, t], po, AF.Copy)
                osrc = bass.AP(
                    tensor=osb.tensor,
                    offset=osb[:, t].offset,
                    ap=[osb.ap[0], [16, 4], [8, 2], [1, 5]],
                )
                odst = bass.AP(
                    tensor=out_sb.tensor,
                    offset=out_sb[:, t].offset,
                    ap=[out_sb.ap[0], [10, 4], [5, 2], [1, 5]],
                )
                nc.scalar.activation(odst, osrc, AF.Tanh)

            ZIN = [zb0, z1, z2]
            ZOUT = [z1, z2, z3]
            WGS = [None, wgs1_sb, wgs2_sb]

            zf1 = ffp.tile([P, NT, 16, P], BF16, tag="f")
            xf1 = ffp.tile([P, NT, 16, P], BF16, tag="f")

            for l in range(3):
                D = DIMS[l]
                E = EDIMS[l]
                kk = KC[l]
                z_in = ZIN[l]
                z_out = ZOUT[l]

                if l == 1:
                    zf, xf = zf1, xf1
                elif l == 2:
                    zf_t = ffp.tile([P, NT, 16, P], BF16, tag="f")
                    xf_t = ffp.tile([P, NT, 16, P], BF16, tag="f")
                    zf = zf_t[:, :, :8]
                    xf = xf_t[:, :, :8]

                # ---- per tile: z transpose, gate matmul, exp (l0's
                # attention is precomputed on host into zb0 = x0) ----
                if l > 0:
                    for t in range(NT):
                        flat = z_in[:, t].rearrange("p a b -> p (a b)")
                        transpose_to(zf[:, t], flat, kk)
                        pg = pgp.tile([P, 64], F32, tag="pg")
                        for c in range(kk):
                            nc.tensor.matmul(
                                pg[:, :56],
                                zf[:, t, c],
                                WGS[l][:, c],
                                start=(c == 0),
                                stop=(c == kk - 1),
                            )
                        nc.scalar.activation(e_sb[:, t], pg[:, :56], AF.Exp)

                xbl = [zb0, xb1, xb2][l]
                HT = NT // 2
                cc4 = cc.rearrange("p t (a b) -> p t a b", a=8)
                for t in range(NT):
                    if l > 0 and t % HT == 0:
                        # ---- softmax coefficients for this half of the
                        # tiles (mix starts before the last tiles' exp) ----
                        ts = slice(t, t + HT)
                        e4 = e_sb[:, ts].rearrange("p t (a b) -> p t a b", a=8)
                        nc.vector.tensor_reduce(
                            s8[:, ts], e4, axis=mybir.AxisListType.X, op=ALU.add
                        )
                        nc.vector.reciprocal(
                            sr8[:, ts].rearrange("p t a -> p (t a)"),
                            s8[:, ts].rearrange("p t a -> p (t a)"),
                        )
                        nc.vector.tensor_tensor(
                            cc[:, ts], e_sb[:, ts], gates_sb[:, ts], op=ALU.mult
                        )
                        nc.vector.tensor_tensor(
                            cc4[:, ts], cc4[:, ts],
                            _bcast_last(sr8[:, ts], 7), op=ALU.mult
                        )
                    if l > 0:
                        # ---- neighbor mix (Pool lacks STT: all DVE) ----
                        for k in range(7):
                            for i in range(8):
                                j = int(NEIGH[i, k])
                                nc.vector.scalar_tensor_tensor(
                                    xbl[:, t, i],
                                    z_in[:, t, j],
                                    cc4[:, t, i, k : k + 1],
                                    z_in[:, t, i] if k == 0 else xbl[:, t, i],
                                    op0=ALU.mult,
                                    op1=ALU.add,
                                )

                    # ---- x transpose ----
                    xflat = xbl[:, t].rearrange("p a b -> p (a b)")
                    if l == 0:
                        pt = ptp.tile([P, P], BF16, tag="pt")
                        nc.tensor.transpose(pt[0 : 8 * D0, :], xflat, ident)
                        copy_eng(xf0[:, t], pt[0 : 8 * D0, :])
                    else:
                        transpose_to(xf[:, t], xflat, kk)

                    # ---- main matmuls into PSUM ----
                    for g in range(2):
                        ph = php.tile([P, 4, 256], F32, tag="ph")
                        for i4 in range(4):
                            i = 4 * g + i4
                            hslice = ph[:, i4, :E]
                            if l == 0:
                                nc.tensor.matmul(
                                    hslice, xf0[:, t], w0_sb[:, i],
                                    start=True, stop=True,
                                )
                            elif l == 1:
                                for c in range(2):
                                    nc.tensor.matmul(
                                        hslice,
                                        xf[:, t, 2 * i + c],
                                        w1_sb[:, i, c],
                                        start=(c == 0),
                                        stop=(c == 1),
                                    )
                            else:
                                nc.tensor.matmul(
                                    hslice, xf[:, t, i], w2_sb[:, i],
                                    start=True, stop=True,
                                )

                        # ---- LN stats for this branch half ----
                        for i4 in range(4):
                            nc.vector.bn_stats(
                                stats[:, t, 4 * g + i4], ph[:, i4, :E]
                            )
                        sg = slice(4 * g, 4 * g + 4)
                        sA = stats[:, t, sg, 1]   # mean of even elements
                        sB = stats[:, t, sg, 4]   # mean of odd elements
                        sC = stats[:, t, sg, 2]   # count*var even
                        sD = stats[:, t, sg, 5]   # count*var odd
                        nc.vector.tensor_tensor(t1v[:, t, sg], sA, sB, op=ALU.add)
                        nc.vector.tensor_tensor(t2v[:, t, sg], sC, sD, op=ALU.add)
                        nc.vector.tensor_tensor(
                            t3v[:, t, sg], sA, sB, op=ALU.subtract
                        )
                        nc.vector.tensor_tensor(
                            t3v[:, t, sg], t3v[:, t, sg], t3v[:, t, sg], op=ALU.mult
                        )
                        nc.vector.tensor_scalar_mul(t2v[:, t, sg], t2v[:, t, sg], 1.0 / E)
                        nc.vector.scalar_tensor_tensor(
                            varv[:, t, sg], t3v[:, t, sg], 0.25, t2v[:, t, sg],
                            op0=ALU.mult, op1=ALU.add,
                        )
                        nc.scalar.activation(
                            stdv[:, t, sg], varv[:, t, sg], AF.Sqrt, bias=eps_t[:, 0:1]
                        )
                        nc.vector.reciprocal(rstd[:, t, sg], stdv[:, t, sg])
                        nc.vector.scalar_tensor_tensor(
                            nmr[:, t, sg], t1v[:, t, sg], -0.5, rstd[:, t, sg],
                            op0=ALU.mult, op1=ALU.mult,
                        )
                        # absorber: ACT picks up the DVE tick before the relus
                        nc.scalar.activation(
                            scr_a if g == 0 else scr_p1,
                            nmr[0:2, t, 4 * g : 4 * g + 2],
                            AF.Copy,
                        )
                        for i4 in range(4):
                            i = 4 * g + i4
                            nc.scalar.activation(
                                z_out[:, t, i],
                                ph[:, i4, :E],
                                AF.Relu,
                                bias=nmr[:, t, i : i + 1],
                                scale=rstd[:, t, i : i + 1],
                            )

                    if l == 2 and t == 3:
                        # interleave the head for the first half of the
                        # tiles under the second half's mix work
                        z3f_t = ffp.tile([P, NT, 16, P], BF16, tag="f")
                        _z3f[0] = z3f_t[:, :, :4]
                        for tt in range(4):
                            head_tile(tt)

            # ---------- output head (tiles 4..7; 0..3 were interleaved
            # into the l2 tile loop to fill DVE idle) ----------
            for t in range(NT // 2, NT):
                head_tile(t)

            # absorb the ACT dependency on the Pool engine so the store DMA
            # carries a single sync wait (hardware limit on DMA structs)
            scr = singles.tile([P, NT], F32)
            nc.gpsimd.tensor_copy(scr, out_sb[:, :, 39])
            nc.gpsimd.dma_start(
                out=out_ext.rearrange("(t p) f -> p t f", p=P), in_=out_sb
            )
    return nc


_NC_CACHE = None


def kernel(**inputs):
    global _NC_CACHE
    f32 = np.float32
    import ml_dtypes

    tobf = lambda x: np.asarray(x, dtype=f32).astype(ml_dtypes.bfloat16)

    Z = np.stack([np.asarray(inputs[f"Z{i}"], f32) for i in range(1, 9)], axis=1)  # [B,8,3]
    gates = np.asarray(inputs["A"], f32)[:, GIDX]                                  # [B,56]

    # ---- layer-0 attention on host (input-dependent preprocessing) ----
    Wg0 = np.asarray(inputs["Wg0"], f32)            # [8, 21, 7]
    bg0 = np.asarray(inputs["bg0"], f32)            # [8, 7]
    Zn0 = Z[:, NEIGH, :]                            # [B, 8, 7, 3]
    Zall0 = Zn0.reshape(B, 8, 21)
    logit0 = np.einsum("bif,ifo->bio", Zall0, Wg0) + bg0
    e0 = np.exp(logit0 - logit0.max(-1, keepdims=True))
    a0 = e0 / e0.sum(-1, keepdims=True)
    c0 = a0 * gates.reshape(B, 8, 7)
    x0 = Z + np.einsum("bik,bikd->bid", c0, Zn0)    # [B, 8, 3]
    z0 = np.zeros((B, 8, D0), f32)
    z0[:, :, :3] = x0
    z0 = z0.reshape(B, 8 * D0)

    DIMS = [3, 256, 128]
    wgs_list = []
    for l in range(1, 3):
        D = DIMS[l]
        Wg = np.asarray(inputs[f"Wg{l}"], f32)      # [8, 7D, 7]
        S = np.zeros((8 * D, 56), f32)
        for i in range(8):
            for k in range(7):
                j = int(NEIGH[i, k])
                S[j * D : (j + 1) * D, i * 7 : (i + 1) * 7] += Wg[i, k * D : (k + 1) * D, :]
        wgs_list.append(S)
    wgs1 = tobf(wgs_list[0].reshape(16, 128, 56))
    wgs2 = tobf(wgs_list[1].reshape(8, 128, 56))

    W0 = np.asarray(inputs["W0"], f32)              # [8,3,256]
    w0 = np.zeros((8, 8 * D0, 256), f32)
    for i in range(8):
        w0[i, D0 * i : D0 * i + 3, :] = W0[i]
    w1 = np.asarray(inputs["W1"], f32).reshape(8, 2, 128, 128)
    w2 = np.asarray(inputs["W2"], f32)              # [8,128,64]
    Wo = np.asarray(inputs["Wout"], f32)            # [8,64,5]
    wo = np.zeros((4, 128, 16), f32)
    for c in range(4):
        wo[c, 0:64, 0:5] = Wo[2 * c]
        wo[c, 64:128, 8:13] = Wo[2 * c + 1]

    ident = np.eye(P, dtype=f32)

    if _NC_CACHE is None:
        _NC_CACHE = build_nc()
    nc = _NC_CACHE

    z0bf = tobf(z0)
    in_maps = []
    for c in range(NCORES):
        sl = slice(c * BL, (c + 1) * BL)
        in_maps.append({
            "z0": z0bf[sl].copy(),
            "gates": gates[sl].copy(),
            "ident": tobf(ident),
            "wgs1": wgs1, "wgs2": wgs2,
            "w0": tobf(w0), "w1": tobf(w1), "w2": tobf(w2), "wo": tobf(wo),
        })
    res = run_bass_kernel_spmd(nc, in_maps, core_ids=list(range(NCORES)))
    outs = [res.results[c]["out"].reshape(BL, 8, 5) for c in range(NCORES)]
    return np.concatenate(outs, axis=0)


# revision 67
# speedup vs baseline: 1.2786x; 1.2786x over previous
"""Trainium2 kernel for nn_ActorNetwork (8-branch graph-attention MLP).

Sharding: pure data parallelism. 8192 samples -> 8 cores x 1024.
Layer-major schedule; all per-tile intermediates resident in SBUF.
Per layer: PE transposes z -> zf (feature-major), gate matmul + batched
softmax, neighbor mix as scalar_tensor_tensor split across DVE and
GpSimd, PE transposes of mixed x -> xf, per-branch main matmuls into
PSUM, LayerNorm stats via bn_stats straight off PSUM, ReLU+affine on
ACT reading PSUM. PSUM->SBUF transpose copies rotate across
ACT/DVE/GpSimd. ACT table thrash avoided by batching exp per layer and
keeping only Copy/Relu (table-free) between the per-tile Sqrt calls.
"""

import numpy as np

import concourse.bass as bass
import concourse.mybir as mybir
import concourse.tile as tile
from concourse.bass_utils import run_bass_kernel_spmd

B = 8192
NCORES = 8
BL = B // NCORES          # 1024 samples per core
P = 128                   # partition tile (samples)
NT = BL // P              # 8 batch tiles per core
HID = [256, 128, 64]
OUT = 5
EPS = 1e-5
D0 = 4                    # layer-0 feature dim padded 3 -> 4

NEIGH = np.array([[j for j in range(8) if j != i] for i in range(8)])
_G = [
    [(0, 1), (0, 2), (0, 3), (4, 0), (5, 0), (6, 0), (7, 0)],
    [(0, 1), (2, 1), (3, 1), (4, 1), (5, 1), (6, 1), (7, 1)],
    [(0, 2), (1, 2), (3, 2), (4, 2), (5, 2), (6, 2), (7, 2)],
    [(0, 3), (1, 3), (2, 3), (4, 3), (5, 3), (6, 3), (7, 3)],
    [(0, 4), (1, 4), (2, 4), (3, 4), (5, 4), (6, 4), (7, 4)],
    [(0, 5), (1, 5), (2, 5), (3, 5), (4, 5), (6, 5), (7, 5)],
    [(0, 6), (1, 6), (2, 6), (3, 6), (4, 6), (5, 6), (7, 6)],
    [(0, 7), (1, 7), (2, 7), (3, 7), (4, 7), (5, 7), (6, 7)],
]
GIDX = np.array([[r * 8 + c for (r, c) in row] for row in _G]).reshape(-1)

F32 = mybir.dt.float32
BF16 = mybir.dt.bfloat16
AF = mybir.ActivationFunctionType
ALU = mybir.AluOpType

MIX_POOL_MIN_I = 4        # branches >= this go to GpSimd in the mix

# The tail drain accumulates one sync-wait per live semaphore lane; walrus
# rejects >1 wait on the Drain struct. Split the waits across SP nops, one
# lane each, before the real drain runs (it then has nothing left to wait on).
_orig_dab = tile.TileContext._drain_and_barrier


def _patched_dab(self, tick_clock, wait_clock):
    from concourse.vector_clock import ScopedClock, VectorClock

    gc = tick_clock.global_clock
    nproc = len(list(gc))
    for proc in range(nproc):
        val = gc[proc]
        if val <= 0:
            continue
        v = VectorClock()
        v.require_at_least(proc, val)
        nop = self.nc.sync.nop(nofuse=True)
        wait_clock.add_sem_waits(nop.ins, ScopedClock({None: v}))
    self.nc.sync.drain()
    self.nc.all_engine_barrier()
    assert self.sems is not None
    popped = self.nc._tile_sem_poison_stack.pop()
    assert popped is self._sem_poison
    self.nc.clear_and_free_semaphores(list(self.sems.allocated().values()))
    self.nc.all_engine_barrier()


tile.TileContext._drain_and_barrier = _patched_dab


def _bcast_last(ap, n):
    """Append a stride-0 innermost dim of size n to an AP (free-dim bcast)."""
    return bass.AP(tensor=ap.tensor, offset=ap.offset, ap=[*ap.ap, [0, n]])


def build_nc():
    nc = bass.Bass()
    DIMS = [D0, 256, 128]            # per-layer input dims (padded l0)
    EDIMS = [256, 128, 64]           # per-layer output dims
    KC = [1, 16, 8]                  # feature-major chunks of z per layer

    # ---- dram parameters ----
    z0_ext = nc.declare_dram_parameter("z0", [BL, 8 * D0], BF16, isOutput=False)
    gates_ext = nc.declare_dram_parameter("gates", [BL, 56], F32, isOutput=False)
    ident_ext = nc.declare_dram_parameter("ident", [P, P], BF16, isOutput=False)
    wgs1_ext = nc.declare_dram_parameter("wgs1", [16, 128, 56], BF16, isOutput=False)
    wgs2_ext = nc.declare_dram_parameter("wgs2", [8, 128, 56], BF16, isOutput=False)
    w0_ext = nc.declare_dram_parameter("w0", [8, 8 * D0, 256], BF16, isOutput=False)
    w1_ext = nc.declare_dram_parameter("w1", [8, 2, 128, 128], BF16, isOutput=False)
    w2_ext = nc.declare_dram_parameter("w2", [8, 128, 64], BF16, isOutput=False)
    wo_ext = nc.declare_dram_parameter("wo", [4, 128, 16], BF16, isOutput=False)
    out_ext = nc.declare_dram_parameter("out", [BL, 40], F32, isOutput=True)

    with tile.TileContext(nc) as tc:
        with (
            tc.tile_pool(name="singles", bufs=1) as singles,
            tc.tile_pool(name="ff", bufs=2) as ffp,
            tc.tile_pool(name="pt", bufs=2, space="PSUM") as ptp,
            tc.tile_pool(name="pg", bufs=2, space="PSUM") as pgp,
            tc.tile_pool(name="ph", bufs=1, space="PSUM") as php,
        ):
            # ---------- constants / weights ----------
            ident = singles.tile([P, P], BF16)
            nc.gpsimd.dma_start(out=ident, in_=ident_ext[:, :])
            eps_t = singles.tile([P, 1], F32)
            nc.vector.memset(eps_t, EPS)

            wgs1_sb = singles.tile([128, 16, 56], BF16)
            nc.gpsimd.dma_start(out=wgs1_sb, in_=wgs1_ext.rearrange("k p f -> p k f"))
            wgs2_sb = singles.tile([128, 8, 56], BF16)
            nc.gpsimd.dma_start(out=wgs2_sb, in_=wgs2_ext.rearrange("k p f -> p k f"))
            w0_sb = singles.tile([8 * D0, 8, 256], BF16)
            nc.gpsimd.dma_start(out=w0_sb, in_=w0_ext.rearrange("i p f -> p i f"))
            w1_sb = singles.tile([128, 8, 2, 128], BF16)
            nc.gpsimd.dma_start(out=w1_sb, in_=w1_ext.rearrange("i k p f -> p i k f"))
            w2_sb = singles.tile([128, 8, 64], BF16)
            nc.gpsimd.dma_start(out=w2_sb, in_=w2_ext.rearrange("i p f -> p i f"))
            wo_sb = singles.tile([128, 4, 16], BF16)
            nc.gpsimd.dma_start(out=wo_sb, in_=wo_ext.rearrange("c p f -> p c f"))

            # ---------- activations ----------
            zb0 = singles.tile([P, NT, 8, D0], BF16)
            nc.gpsimd.dma_start(
                out=zb0.rearrange("p t a b -> p t (a b)"),
                in_=z0_ext.rearrange("(t p) f -> p t f", p=P),
            )
            gates_sb = singles.tile([P, NT, 56], F32)
            nc.gpsimd.dma_start(
                out=gates_sb, in_=gates_ext.rearrange("(t p) f -> p t f", p=P)
            )

            z1 = singles.tile([P, NT, 8, 256], BF16)
            z2 = singles.tile([P, NT, 8, 128], BF16)
            z3 = singles.tile([P, NT, 8, 64], BF16)
            xf0 = singles.tile([8 * D0, NT, P], BF16)
            xb1 = singles.tile([P, NT, 8, 256], BF16)
            xb2 = singles.tile([P, NT, 8, 128], BF16)

            e_sb = singles.tile([P, NT, 56], F32)
            cc = singles.tile([P, NT, 56], F32)
            s8 = singles.tile([P, NT, 8], F32)
            sr8 = singles.tile([P, NT, 8], F32)
            stats = singles.tile([P, NT, 8, 6], F32)
            t1v = singles.tile([P, NT, 8], F32)
            t2v = singles.tile([P, NT, 8], F32)
            t3v = singles.tile([P, NT, 8], F32)
            varv = singles.tile([P, NT, 8], F32)
            stdv = singles.tile([P, NT, 8], F32)
            rstd = singles.tile([P, NT, 8], F32)
            nmr = singles.tile([P, NT, 8], F32)
            scr_a = singles.tile([2, 2], F32)
            scr_p1 = singles.tile([2, 2], F32)
            scr_p2 = singles.tile([2, 2], BF16)
            scr_v = singles.tile([2, 2], BF16)
            osb = singles.tile([P, NT, 64], F32)
            out_sb = singles.tile([P, NT, 40], F32)

            _eng_ctr = [0]

            def copy_eng(dst, src):
                # All PSUM->SBUF copies on ACT so every consumer's psum-WAR
                # and copy-RAW deps collapse onto the single Activation lane.
                nc.scalar.activation(dst, src, AF.Copy)

            def transpose_to(dstf, flat, ncols):
                """PE-transpose flat [128, ncols*128] into dstf chunks
                [ncols, 128]: per-chunk psum tile + ACT copy (the exact
                structure the scheduler handles within its one-wait-per-
                instruction budget)."""
                for c in range(ncols):
                    pt = ptp.tile([P, P], BF16, tag="pt")
                    nc.tensor.transpose(pt, flat[:, c * P : (c + 1) * P], ident)
                    copy_eng(dstf[:, c], pt)

            _z3f = [None]

            def head_tile(t):
                z3f = _z3f[0]
                flat = z3[:, t].rearrange("p a b -> p (a b)")
                transpose_to(z3f[:, t], flat, 4)
                po = pgp.tile([P, 64], F32, tag="pg")
                for c in range(4):
                    nc.tensor.matmul(
                        po[:, c * 16 : (c + 1) * 16],
                        z3f[:, t, c],
                        wo_sb[:, c],
                        start=True,
                        stop=True,
                    )
                nc.scalar.activation(osb[:, t], po, AF.Copy)
                osrc = bass.AP(
                    tensor=osb.tensor,
                    offset=osb[:, t].offset,
                    ap=[osb.ap[0], [16, 4], [8, 2], [1, 5]],
                )
                odst = bass.AP(
                    tensor=out_sb.tensor,
                    offset=out_sb[:, t].offset,
                    ap=[out_sb.ap[0], [10, 4], [5, 2], [1, 5]],
                )
                nc.scalar.activation(odst, osrc, AF.Tanh)

            ZIN = [zb0, z1, z2]
            ZOUT = [z1, z2, z3]
            WGS = [None, wgs1_sb, wgs2_sb]

            zf1 = ffp.tile([P, NT, 16, P], BF16, tag="f")
            xf1 = ffp.tile([P, NT, 16, P], BF16, tag="f")

            for l in range(3):
                D = DIMS[l]
                E = EDIMS[l]
                kk = KC[l]
                z_in = ZIN[l]
                z_out = ZOUT[l]

                if l == 1:
                    zf, xf = zf1, xf1
                elif l == 2:
                    zf_t = ffp.tile([P, NT, 16, P], BF16, tag="f")
                    xf_t = ffp.tile([P, NT, 16, P], BF16, tag="f")
                    zf = zf_t[:, :, :8]
                    xf = xf_t[:, :, :8]

                # ---- per tile: z transpose, gate matmul, exp (l0's
                # attention is precomputed on host into zb0 = x0) ----
                if l > 0:
                    for t in range(NT):
                        flat = z_in[:, t].rearrange("p a b -> p (a b)")
                        transpose_to(zf[:, t], flat, kk)
                        pg = pgp.tile([P, 64], F32, tag="pg")
                        for c in range(kk):
                            nc.tensor.matmul(
                                pg[:, :56],
                                zf[:, t, c],
                                WGS[l][:, c],
                                start=(c == 0),
                                stop=(c == kk - 1),
                            )
                        nc.scalar.activation(e_sb[:, t], pg[:, :56], AF.Exp)

                xbl = [zb0, xb1, xb2][l]
                HT = NT // 4
                cc4 = cc.rearrange("p t (a b) -> p t a b", a=8)
                for t in range(NT):
                    if l > 0 and t % HT == 0:
                        # ---- softmax coefficients for this half of the
                        # tiles (mix starts before the last tiles' exp) ----
                        ts = slice(t, t + HT)
                        e4 = e_sb[:, ts].rearrange("p t (a b) -> p t a b", a=8)
                        nc.vector.tensor_reduce(
                            s8[:, ts], e4, axis=mybir.AxisListType.X, op=ALU.add
                        )
                        nc.vector.reciprocal(
                            sr8[:, ts].rearrange("p t a -> p (t a)"),
                            s8[:, ts].rearrange("p t a -> p (t a)"),
                        )
                        nc.vector.tensor_tensor(
                            cc[:, ts], e_sb[:, ts], gates_sb[:, ts], op=ALU.mult
                        )
                        nc.vector.tensor_tensor(
                            cc4[:, ts], cc4[:, ts],
                            _bcast_last(sr8[:, ts], 7), op=ALU.mult
                        )
                    if l > 0:
                        # ---- neighbor mix (Pool lacks STT: all DVE) ----
                        for k in range(7):
                            for i in range(8):
                                j = int(NEIGH[i, k])
                                nc.vector.scalar_tensor_tensor(
                                    xbl[:, t, i],
                                    z_in[:, t, j],
                                    cc4[:, t, i, k : k + 1],
                                    z_in[:, t, i] if k == 0 else xbl[:, t, i],
                                    op0=ALU.mult,
                                    op1=ALU.add,
                                )

                    # ---- x transpose ----
                    xflat = xbl[:, t].rearrange("p a b -> p (a b)")
                    if l == 0:
                        pt = ptp.tile([P, P], BF16, tag="pt")
                        nc.tensor.transpose(pt[0 : 8 * D0, :], xflat, ident)
                        copy_eng(xf0[:, t], pt[0 : 8 * D0, :])
                    else:
                        transpose_to(xf[:, t], xflat, kk)

                    # ---- main matmuls into PSUM ----
                    ph = php.tile([P, 8, 256], F32, tag="ph")
                    for i in range(8):
                        hslice = ph[:, i, :E]
                        if l == 0:
                            nc.tensor.matmul(
                                hslice, xf0[:, t], w0_sb[:, i], start=True, stop=True
                            )
                        elif l == 1:
                            for c in range(2):
                                nc.tensor.matmul(
                                    hslice,
                                    xf[:, t, 2 * i + c],
                                    w1_sb[:, i, c],
                                    start=(c == 0),
                                    stop=(c == 1),
                                )
                        else:
                            nc.tensor.matmul(
                                hslice, xf[:, t, i], w2_sb[:, i], start=True, stop=True
                            )

                    # ---- LN stats ----
                    for i in range(8):
                        nc.vector.bn_stats(stats[:, t, i], ph[:, i, :E])
                    sA = stats[:, t, :, 1]   # mean of even elements
                    sB = stats[:, t, :, 4]   # mean of odd elements
                    sC = stats[:, t, :, 2]   # count*var even
                    sD = stats[:, t, :, 5]   # count*var odd
                    nc.vector.tensor_tensor(t1v[:, t], sA, sB, op=ALU.add)
                    nc.vector.tensor_tensor(t2v[:, t], sC, sD, op=ALU.add)
                    nc.vector.tensor_tensor(t3v[:, t], sA, sB, op=ALU.subtract)
                    nc.vector.tensor_tensor(t3v[:, t], t3v[:, t], t3v[:, t], op=ALU.mult)
                    nc.vector.tensor_scalar_mul(t2v[:, t], t2v[:, t], 1.0 / E)
                    nc.vector.scalar_tensor_tensor(
                        varv[:, t], t3v[:, t], 0.25, t2v[:, t],
                        op0=ALU.mult, op1=ALU.add,
                    )
                    nc.scalar.activation(
                        stdv[:, t], varv[:, t], AF.Sqrt, bias=eps_t[:, 0:1]
                    )
                    nc.vector.reciprocal(rstd[:, t], stdv[:, t])
                    nc.vector.scalar_tensor_tensor(
                        nmr[:, t], t1v[:, t], -0.5, rstd[:, t],
                        op0=ALU.mult, op1=ALU.mult,
                    )
                    # absorber: ACT picks up the DVE tick before the relus
                    nc.scalar.activation(scr_a, nmr[0:2, t, 0:2], AF.Copy)
                    for i in range(8):
                        nc.scalar.activation(
                            z_out[:, t, i],
                            ph[:, i, :E],
                            AF.Relu,
                            bias=nmr[:, t, i : i + 1],
                            scale=rstd[:, t, i : i + 1],
                        )

                    if l == 2 and t == 3:
                        # interleave the head for the first half of the
                        # tiles under the second half's mix work
                        z3f_t = ffp.tile([P, NT, 16, P], BF16, tag="f")
                        _z3f[0] = z3f_t[:, :, :4]
                        for tt in range(4):
                            head_tile(tt)

            # ---------- output head (tiles 4..7; 0..3 were interleaved
            # into the l2 tile loop to fill DVE idle) ----------
            for t in range(NT // 2, NT):
                head_tile(t)

            # absorb the ACT dependency on the Pool engine so the store DMA
            # carries a single sync wait (hardware limit on DMA structs)
            scr = singles.tile([P, NT], F32)
            nc.gpsimd.tensor_copy(scr, out_sb[:, :, 39])
            nc.gpsimd.dma_start(
                out=out_ext.rearrange("(t p) f -> p t f", p=P), in_=out_sb
            )
    return nc


_NC_CACHE = None


def kernel(**inputs):
    global _NC_CACHE
    f32 = np.float32
    import ml_dtypes

    tobf = lambda x: np.asarray(x, dtype=f32).astype(ml_dtypes.bfloat16)

    Z = np.stack([np.asarray(inputs[f"Z{i}"], f32) for i in range(1, 9)], axis=1)  # [B,8,3]
    gates = np.asarray(inputs["A"], f32)[:, GIDX]                                  # [B,56]

    # ---- layer-0 attention on host (input-dependent preprocessing) ----
    Wg0 = np.asarray(inputs["Wg0"], f32)            # [8, 21, 7]
    bg0 = np.asarray(inputs["bg0"], f32)            # [8, 7]
    Zn0 = Z[:, NEIGH, :]                            # [B, 8, 7, 3]
    Zall0 = Zn0.reshape(B, 8, 21)
    logit0 = np.einsum("bif,ifo->bio", Zall0, Wg0) + bg0
    e0 = np.exp(logit0 - logit0.max(-1, keepdims=True))
    a0 = e0 / e0.sum(-1, keepdims=True)
    c0 = a0 * gates.reshape(B, 8, 7)
    x0 = Z + np.einsum("bik,bikd->bid", c0, Zn0)    # [B, 8, 3]
    z0 = np.zeros((B, 8, D0), f32)
    z0[:, :, :3] = x0
    z0 = z0.reshape(B, 8 * D0)

    DIMS = [3, 256, 128]
    wgs_list = []
    for l in range(1, 3):
        D = DIMS[l]
        Wg = np.asarray(inputs[f"Wg{l}"], f32)      # [8, 7D, 7]
        S = np.zeros((8 * D, 56), f32)
        for i in range(8):
            for k in range(7):
                j = int(NEIGH[i, k])
                S[j * D : (j + 1) * D, i * 7 : (i + 1) * 7] += Wg[i, k * D : (k + 1) * D, :]
        wgs_list.append(S)
    wgs1 = tobf(wgs_list[0].reshape(16, 128, 56))
    wgs2 = tobf(wgs_list[1].reshape(8, 128, 56))

    W0 = np.asarray(inputs["W0"], f32)              # [8,3,256]
    w0 = np.zeros((8, 8 * D0, 256), f32)
    for i in range(8):
        w0[i, D0 * i : D0 * i + 3, :] = W0[i]
    w1 = np.asarray(inputs["W1"], f32).reshape(8, 2, 128, 128)
    w2 = np.asarray(inputs["W2"], f32)              # [8,128,64]
    Wo = np.asarray(inputs["Wout"], f32)            # [8,64,5]
    wo = np.zeros((4, 128, 16), f32)
    for c in range(4):
        wo[c, 0:64, 0:5] = Wo[2 * c]
        wo[c, 64:128, 8:13] = Wo[2 * c + 1]

    ident = np.eye(P, dtype=f32)

    if _NC_CACHE is None:
        _NC_CACHE = build_nc()
    nc = _NC_CACHE

    z0bf = tobf(z0)
    in_maps = []
    for c in range(NCORES):
        sl = slice(c * BL, (c + 1) * BL)
        in_maps.append({
            "z0": z0bf[sl].copy(),
            "gates": gates[sl].copy(),
            "ident": tobf(ident),
            "wgs1": wgs1, "wgs2": wgs2,
            "w0": tobf(w0), "w1": tobf(w1), "w2": tobf(w2), "wo": tobf(wo),
        })
    res = run_bass_kernel_spmd(nc, in_maps, core_ids=list(range(NCORES)))
    outs = [res.results[c]["out"].reshape(BL, 8, 5) for c in range(NCORES)]
    return np.concatenate(outs, axis=0)
